# revision 13
# baseline (speedup 1.0000x reference)
"""Batched int8 GEMM with scaling for TRN2: out[b] = round(alpha * (a[b] @ b[b]^T)).

Shapes (hardcoded per the problem spec): a [64,1024,128] int8, b [64,1024,128] int8,
alpha fp32 scalar -> out [64,1024,1024] int32.

Strategy:
- Shard batch dim B=64 across 8 NeuronCores (8 batches/core), no communication.
- Host-side prep: transpose to a^T [B,K,M] / b^T [B,K,N] (K=128 on partitions, the
  layout the PE array needs for both operands). int8 -> bf16 cast happens inside the
  SWDGE DMA (exact for [-128,127]); K=128-deep dot products are exact in the fp32
  PSUM accumulator, so the GEMM is bit-exact.
- Fine-grained m-tile pipeline: per 128-row m-tile, two 128x128x512 matmuls into a
  2-bank PSUM tile, one fused epilogue op (mul-by-alpha + fp32->int cast, round-to-
  nearest-even = jnp.round) alternating VectorE/ScalarE, then a 256KB fully-contiguous
  output DMA alternating the sync/gpsimd queues (ScalarE keeps epilogue duty; it only
  takes over gpsimd's output chunks near the end so the SWDGE FIFO drains early).
- Input cast-DMAs are dispatched just-in-time (two batches of lead) so the gpsimd
  queue is not clogged at t=0; batch 0's loads are split (b half / first a m-tile
  first) so the first matmul fires as early as possible.
- Device output is int16 when alpha bounds |out| < 32768 (true for alpha=2^-7:
  |acc| <= 2^21 -> |out| <= 16384), halving the dominant HBM write traffic; host
  upcasts to int32. Output DRAM layout [batch, m-tile, row, N] makes every chunk a
  contiguous 256KB block and host un-tiling a pure reshape.
"""

import sys

sys.path.insert(0, "/opt/trn_rl_repo")

from contextlib import ExitStack

import numpy as np

import concourse.tile as tile
from concourse import bacc, mybir
from concourse.bass_utils import run_bass_kernel_spmd

B, M, N, K = 64, 1024, 1024, 128
N_CORES = 8
BPC = B // N_CORES  # batches per core
MT = 128  # m-tile (PSUM partition dim)
NT = 512  # n-tile (one PSUM bank of fp32)
NMT = M // MT  # m-tiles per batch

ACC_MAX = 128 * 128 * K  # max |a@b^T| entry for int8 operands

_cache: dict = {}


def _build(alpha: float, out16: bool):
    out_dt = mybir.dt.int16 if out16 else mybir.dt.int32
    nc = bacc.Bacc(
        "TRN2", target_bir_lowering=False, debug=False, num_devices=N_CORES
    )
    # int8 inputs, upcast to bf16 during the SWDGE DMA (halves input HBM
    # traffic; HWDGE cannot cast, so all input loads ride the gpsimd queue).
    aT = nc.dram_tensor("aT", [BPC, K, M], mybir.dt.int8, kind="ExternalInput").ap()
    bT = nc.dram_tensor("bT", [BPC, K, N], mybir.dt.int8, kind="ExternalInput").ap()
    # m-tile-major output layout [batch, m-tile, row-in-tile, n]: each output
    # chunk is one fully contiguous 256KB block in DRAM (longest HBM bursts)
    # and the host un-tile is a plain reshape.
    out_r = nc.dram_tensor(
        "out", [BPC, NMT, MT, N], out_dt, kind="ExternalOutput"
    ).ap()

    with tile.TileContext(nc) as tc, ExitStack() as ctx:
        a_pool = ctx.enter_context(tc.tile_pool(name="a", bufs=1))
        b_pool = ctx.enter_context(tc.tile_pool(name="b", bufs=1))
        ps_pool = ctx.enter_context(tc.tile_pool(name="ps", bufs=4, space="PSUM"))
        o_pool = ctx.enter_context(tc.tile_pool(name="o", bufs=8))

        # All 8 batches stay resident in SBUF (4KB/partition total); tiles are
        # created up-front, loads dispatched just-in-time in the batch loop.
        ats = [
            a_pool.tile([K, M], mybir.dt.bfloat16, name=f"at{i}", tag=f"a{i}")
            for i in range(BPC)
        ]
        bts = [
            b_pool.tile([K, N], mybir.dt.bfloat16, name=f"bt{i}", tag=f"b{i}")
            for i in range(BPC)
        ]

        def load_batch(i):
            if i == 0:
                # split so the first matmul (needs b[:, :512] + a's first
                # m-tile) waits on as little Q7 descriptor-gen as possible
                nc.gpsimd.dma_start(bts[0][:, :NT], bT[0][:, :NT])
                nc.gpsimd.dma_start(ats[0][:, :MT], aT[0][:, :MT])
                nc.gpsimd.dma_start(bts[0][:, NT:], bT[0][:, NT:])
                nc.gpsimd.dma_start(ats[0][:, MT:], aT[0][:, MT:])
            else:
                # b first: batch i's first matmul needs all of b but only the
                # leading m-tile slice of a
                nc.gpsimd.dma_start(bts[i][:], bT[i])  # int8 -> bf16 in DMA
                nc.gpsimd.dma_start(ats[i][:], aT[i])

        load_batch(0)
        load_batch(1)

        tile_idx = 0
        n_tiles = BPC * NMT
        for i in range(BPC):
            if i + 2 < BPC:
                load_batch(i + 2)  # two batches of lead time
            at, bt = ats[i], bts[i]
            for m in range(NMT):
                ps = ps_pool.tile([MT, N], mybir.dt.float32)
                for n in range(N // NT):
                    nc.tensor.matmul(
                        ps[:, n * NT : (n + 1) * NT],
                        at[:, m * MT : (m + 1) * MT],
                        bt[:, n * NT : (n + 1) * NT],
                        start=True,
                        stop=True,
                    )
                ot = o_pool.tile([MT, N], out_dt)
                # fused scale + fp32->int cast (round-to-nearest-even), one op
                # per m-tile, alternating the two PSUM-capable engines
                if tile_idx % 2 == 0:
                    nc.vector.tensor_scalar_mul(ot[:], ps[:], alpha)
                else:
                    nc.scalar.mul(ot[:], ps[:], alpha)
                # 256KB contiguous chunk; alternate sync/gpsimd queues, but
                # hand gpsimd's tail chunks to scalar so the SWDGE FIFO is
                # empty well before the end (its exit drain is expensive)
                if tile_idx % 2 == 0:
                    eng = nc.sync
                elif n_tiles - tile_idx <= 6:
                    eng = nc.scalar
                else:
                    eng = nc.gpsimd
                eng.dma_start(out_r[i][m], ot[:])
                tile_idx += 1

    nc.compile()
    return nc


def _get(alpha: float, out16: bool):
    key = (alpha, out16)
    if key not in _cache:
        _cache[key] = _build(alpha, out16)
    return _cache[key]


def make_in_maps(a: np.ndarray, b: np.ndarray):
    aT = np.ascontiguousarray(a.transpose(0, 2, 1))
    bT = np.ascontiguousarray(b.transpose(0, 2, 1))
    in_maps = []
    for c in range(N_CORES):
        in_maps.append(
            {
                "aT": aT[c * BPC : (c + 1) * BPC],
                "bT": bT[c * BPC : (c + 1) * BPC],
            }
        )
    return in_maps


def kernel(a: np.ndarray, b: np.ndarray, alpha: np.ndarray) -> np.ndarray:
    alpha_f = float(np.asarray(alpha))
    out16 = abs(alpha_f) * ACC_MAX < 32767.5

    nc = _get(alpha_f, out16)
    in_maps = make_in_maps(a, b)
    res = run_bass_kernel_spmd(nc, in_maps, list(range(N_CORES))).results
    # [BPC, NMT, MT, N] -> [BPC, M, N]: rows are already in order, pure reshape
    out = np.concatenate([res[c]["out"] for c in range(N_CORES)], axis=0)
    out = out.reshape(B, M, N)
    return out.astype(np.int32)
